# revision 1
# baseline (speedup 1.0000x reference)
"""Trainium2 Bass kernel for nn_MultiHeadAttention_34144990003301 (v4).

Head-parallel attention (2 heads/core, BN stats local), bf16 datapath.
BN1 is applied to the attention output locally (batched tensor_scalar),
then flat is AllGathered in bf16 with tokens innermost so the phase-F
matmul moving operand has contiguous columns (PE 4x bf16 column packing).
Each core computes a 640-wide shard of the (zero-padded to 5120)
para_linear1 output for all 512 tokens out of SBUF, leaky relu via
affine + max, partial W2 products AllReduced, sigmoid on device.

kernel(**inputs) takes the full unsharded inputs, returns [32,1,16,64] f32.
"""

import numpy as np

BS, HEADS, FN, SL, KN, ST = 32, 16, 124, 256, 64, 4
HID = 5000
HIDP = 5120                    # zero-padded hid
EPS = 1e-5
SLOPE = 0.01
N_CORES = 8
HL = HEADS // N_CORES          # 2 local heads per core
ROWS = HL * KN                 # 128 projected rows (per-head 64, duplicated)
TL = BS * HL                   # 64 local tokens
T = BS * HEADS                 # 512 global tokens
HSH = HIDP // N_CORES          # 640 hid cols per core (5 psum tiles of 128)
IC = SL // 128                 # 2 i-chunks
NKT = SL * KN // 128           # 128 contraction tiles for para_linear1
FW = IC * KN * TL              # 8192 cols in floc, layout (ic, k, t)
NPRE = 28                      # prefetched W1 kt-tiles (of 128)
# packed f32 const columns
PC_BQ, PC_BK, PC_BV = 0, 1, 2
PC_MASK = 3                    # 2 cols
PC_B1 = 5                      # 5 cols
PC_B2 = 10
PC_ONES = 11
PC_SEL = 12                    # 128 cols (rows 0:2)
PC_BC1 = 140                   # 128 cols (row 0)
PC_BNP = 268                   # 8 cols (rows 0:2)
PC_BNP1 = 276                  # 4 cols (row 0)
PCW = 280

_prog = None


def _build():
    import concourse.bacc as bacc
    import concourse.tile as tile
    import concourse.mybir as mybir

    f32 = mybir.dt.float32
    bf16 = mybir.dt.bfloat16
    AF = mybir.ActivationFunctionType
    OP = mybir.AluOpType
    RG = [list(range(N_CORES))]

    nc = bacc.Bacc("TRN2", target_bir_lowering=False, debug=False,
                   num_devices=N_CORES)

    def din(name, shape, dt=f32):
        return nc.dram_tensor(
            name, list(shape), dt, kind="ExternalInput"
        ).ap()

    q_d = din("qh", (FN, BS * SL), bf16)
    k_d = din("kh", (FN, BS * SL), bf16)
    v_d = din("vh", (FN, BS * SL), bf16)
    wq_d = din("wqT", (FN, ROWS), bf16)
    wk_d = din("wkT", (FN, ROWS), bf16)
    wv_d = din("wvT", (FN, ROWS), bf16)
    pk_d = din("packf", (128, PCW))
    pb_d = din("packb", (128, 128 + 5 * KN), bf16)   # eye128 | w2 tiles
    w1_d = din("w1T", (NKT, 128, HSH), bf16)
    out_d = nc.dram_tensor("out", [KN, T], f32, kind="ExternalOutput").ap()

    with tile.TileContext(nc) as tc:
        with (
            tc.tile_pool(name="persist", bufs=1) as pp,
            tc.tile_pool(name="dram", bufs=1, space="DRAM") as dp,
        ):
            # ---------- input + const + W1-prefetch DMAs ----------
            pk_sb = pp.tile([128, PCW], f32, tag="packf")
            pb_sb = pp.tile([128, 128 + 5 * KN], bf16, tag="packb")
            w1pre = pp.tile([128, NPRE * HSH], bf16, tag="w1pre")

            bq_sb = pk_sb[:, PC_BQ:PC_BQ + 1]
            bk_sb = pk_sb[:, PC_BK:PC_BK + 1]
            bv_sb = pk_sb[:, PC_BV:PC_BV + 1]
            mask_sb = pk_sb[:, PC_MASK:PC_MASK + 2]
            b1_sb = [pk_sb[:, PC_B1 + j:PC_B1 + j + 1] for j in range(5)]
            b2_sb = pk_sb[0:KN, PC_B2:PC_B2 + 1]
            ones128 = pk_sb[:, PC_ONES:PC_ONES + 1]      # value 1/128
            sel_sb = pk_sb[0:HL, PC_SEL:PC_SEL + 128]
            bc1_sb = pk_sb[0:1, PC_BC1:PC_BC1 + 128]
            bnp_sb = pk_sb[0:HL, PC_BNP:PC_BNP + 8]
            bnp1_sb = pk_sb[0:1, PC_BNP1:PC_BNP1 + 4]
            eye_sb = pb_sb[:, 0:128]
            w2_sb = [pb_sb[:, 128 + j * KN:128 + (j + 1) * KN]
                     for j in range(5)]

            # attention output, layout [p=i_in_ic, (ic, k, tl)], tl=hl*32+b
            O_all = pp.tile([128, FW], bf16, tag="oall")

            otp_cm = tc.tile_pool(name="otmp", bufs=1)
            otp = otp_cm.__enter__()
            O_tmp = otp.tile([128, FW], bf16, tag="otmp", name="O_tmp")
            Ot_v = O_tmp[:].rearrange("p (a t k) -> p a t k", a=IC, t=TL)
            with tc.tile_pool(name="proj", bufs=1) as prp:
                qp = prp.tile([ROWS, BS * SL], bf16, tag="qp")
                kp = prp.tile([ROWS, BS * SL], bf16, tag="kp")
                vp = prp.tile([ROWS, BS * SL], bf16, tag="vp")

                # ---------- Phase A: qp/kp/vp = W[R,:] @ x + b ----------
                with (
                    tc.tile_pool(name="xin", bufs=1) as xp,
                    tc.tile_pool(name="psA", bufs=3, space="PSUM") as psA,
                    tc.tile_pool(name="stat", bufs=1) as st,
                ):
                    nc.sync.dma_start(pk_sb[:], pk_d)
                    xw = []
                    for ti, (x_d, w_d) in enumerate(
                        ((q_d, wq_d), (k_d, wk_d), (v_d, wv_d))
                    ):
                        x_sb = xp.tile([FN, BS * SL], bf16, tag=f"x{ti}",
                                       name=f"x{ti}")
                        nc.sync.dma_start(x_sb[:], x_d)
                        w_sb = xp.tile([FN, ROWS], bf16, tag=f"w{ti}",
                                       name=f"w{ti}")
                        nc.sync.dma_start(w_sb[:], w_d)
                        xw.append((x_sb, w_sb))
                    nc.scalar.dma_start(pb_sb[:], pb_d)
                    nc.scalar.dma_start(
                        w1pre[:].rearrange("p (m j) -> p m j", m=NPRE),
                        w1_d[0:NPRE].transpose([1, 0, 2]),
                    )
                    bnsts = [
                        st.tile([ROWS, 16 * 6], f32, tag=f"bnst{ti}",
                                name=f"bnst{ti}")
                        for ti in range(3)
                    ]
                    for ti, (b_sb, dst) in enumerate(
                        ((bq_sb, qp), (bk_sb, kp), (bv_sb, vp))
                    ):
                        x_sb, w_sb = xw[ti]
                        for n in range(16):
                            cs = slice(n * 512, (n + 1) * 512)
                            ps = psA.tile([ROWS, 512], f32, tag="proj",
                                          name=f"proj{ti}_{n}")
                            nc.tensor.matmul(ps[:], w_sb[:], x_sb[:, cs])
                            nc.scalar.activation(
                                dst[:, cs], ps[:], AF.Identity,
                                bias=b_sb, scale=1.0,
                            )
                            nc.vector.bn_stats(
                                bnsts[ti][:, 6 * n:6 * (n + 1)], dst[:, cs]
                            )

                    # ---------- Phase B: per-head BN affine for q/k/v ----
                    with tc.tile_pool(name="psB", bufs=1,
                                      space="PSUM") as psB:
                        AB = st.tile([HL, 6], f32, tag="AB")
                        for ti in range(3):
                            gc, bc_ = 2 * ti, 2 * ti + 1
                            mv = st.tile([ROWS, 2], f32, tag=f"mv{ti}",
                                         name=f"mv{ti}")
                            nc.vector.bn_aggr(
                                mv[:],
                                bnsts[ti][:].rearrange(
                                    "p (c s) -> p c s", s=6
                                ),
                            )
                            stat2 = st.tile([ROWS, 2], f32, tag=f"s2{ti}",
                                            name=f"s2{ti}")
                            nc.vector.tensor_copy(stat2[:, 0:1], mv[:, 0:1])
                            nc.vector.scalar_tensor_tensor(
                                stat2[:, 1:2], mv[:, 0:1], mv[:, 0:1],
                                mv[:, 1:2], op0=OP.mult, op1=OP.add,
                            )
                            hs = psB.tile([HL, 2], f32, tag=f"hs{ti}",
                                          name=f"hs{ti}")
                            nc.tensor.matmul(hs[:], mask_sb, stat2[:])
                            mean_h = st.tile([HL, 1], f32, tag=f"mh{ti}",
                                             name=f"mh{ti}")
                            nc.vector.tensor_copy(mean_h[:], hs[:, 0:1])
                            tmp = st.tile([HL, 1], f32, tag=f"tp{ti}",
                                          name=f"tp{ti}")
                            nc.vector.tensor_tensor(
                                tmp[:], mean_h[:], mean_h[:], op=OP.mult
                            )
                            var_h = st.tile([HL, 1], f32, tag=f"vh{ti}",
                                            name=f"vh{ti}")
                            nc.vector.tensor_tensor(
                                var_h[:], hs[:, 1:2], tmp[:],
                                op=OP.subtract,
                            )
                            nc.vector.tensor_scalar_add(
                                var_h[:], var_h[:], EPS
                            )
                            rv = st.tile([HL, 1], f32, tag=f"rv{ti}",
                                         name=f"rv{ti}")
                            nc.vector.reciprocal(rv[:], var_h[:])
                            rsq = st.tile([HL, 1], f32, tag=f"rq{ti}",
                                          name=f"rq{ti}")
                            nc.scalar.sqrt(rsq[:], rv[:])
                            a_h = st.tile([HL, 1], f32, tag=f"ah{ti}",
                                          name=f"ah{ti}")
                            nc.vector.tensor_tensor(
                                a_h[:], bnp_sb[:, gc:gc + 1], rsq[:],
                                op=OP.mult,
                            )
                            tmp2 = st.tile([HL, 1], f32, tag=f"t2{ti}",
                                           name=f"t2{ti}")
                            nc.vector.tensor_tensor(
                                tmp2[:], mean_h[:], a_h[:], op=OP.mult
                            )
                            nc.vector.tensor_tensor(
                                AB[:, bc_:bc_ + 1], bnp_sb[:, bc_:bc_ + 1],
                                tmp2[:], op=OP.subtract,
                            )
                            nc.vector.tensor_copy(AB[:, gc:gc + 1], a_h[:])
                        bc_ps = psB.tile([128, 6], f32, tag="bcps")
                        nc.tensor.matmul(bc_ps[:], sel_sb, AB[:])
                        ab_sb = pp.tile([128, 6], f32, tag="absb")
                        nc.vector.tensor_copy(ab_sb[:], bc_ps[:])

                # ---------- Phase C: attention, 2 heads per b ----------
                O_v = O_all[:].rearrange("p (a k t) -> p a k t",
                                         a=IC, k=KN)
                with (
                    tc.tile_pool(name="stage", bufs=3) as sg,
                    tc.tile_pool(name="expp", bufs=2) as epool,
                    tc.tile_pool(name="vwp", bufs=3) as vwp,
                    tc.tile_pool(name="small", bufs=4) as smp,
                    tc.tile_pool(name="ps_sc", bufs=2, space="PSUM") as pssc,
                    tc.tile_pool(name="ps_vt", bufs=2, space="PSUM") as psvt,
                    tc.tile_pool(name="ps_uo", bufs=2, space="PSUM") as psuo,
                ):
                    for b in range(BS):
                        bsl = slice(b * SL, (b + 1) * SL)
                        qw2 = sg.tile([128, SL], bf16, tag="qw")
                        nc.gpsimd.tensor_scalar(
                            qw2[:], qp[:, bsl], ab_sb[:, 0:1], ab_sb[:, 1:2],
                            op0=OP.mult, op1=OP.add,
                        )
                        kw2 = sg.tile([128, SL], bf16, tag="kw")
                        nc.gpsimd.tensor_scalar(
                            kw2[:], kp[:, bsl], ab_sb[:, 2:3], ab_sb[:, 3:4],
                            op0=OP.mult, op1=OP.add,
                        )
                        vw2 = sg.tile([128, SL], bf16, tag="vw")
                        nc.vector.tensor_scalar(
                            vw2[:], vp[:, bsl], ab_sb[:, 4:5], ab_sb[:, 5:6],
                            op0=OP.mult, op1=OP.add,
                        )
                        # scores both heads: [128(j in jc), hl*512 + i]
                        sc_ps = pssc.tile([128, 1024], f32, tag="scps")
                        for hl in range(HL):
                            r = slice(KN * hl, KN * (hl + 1))
                            for jc in range(2):
                                nc.tensor.matmul(
                                    sc_ps[:, hl * 512 + jc * 256:
                                          hl * 512 + (jc + 1) * 256],
                                    kw2[r, jc * 128:(jc + 1) * 128],
                                    qw2[r, :],
                                )
                        eT = epool.tile([128, 1024], bf16, tag="expT")
                        for hl in range(HL):
                            h5 = slice(hl * 512, (hl + 1) * 512)
                            nc.scalar.activation(
                                eT[:, h5], sc_ps[:, h5], AF.Exp,
                                bias=0.0, scale=0.125,
                            )
                        # vw transposed: [128(s in jc), k both heads]
                        vt_ps = psvt.tile([128, 256], bf16, tag="vtps")
                        for jc in range(2):
                            nc.tensor.transpose(
                                vt_ps[:, jc * 128:(jc + 1) * 128],
                                vw2[:, jc * 128:(jc + 1) * 128],
                                eye_sb,
                            )
                        vws2 = vwp.tile([128, 2 * 2 * (KN + 1)], bf16,
                                        tag="vws")
                        vws2v = vws2[:].rearrange(
                            "p (a h e) -> p a h e", a=2, h=2
                        )
                        for jc in range(2):
                            nc.vector.tensor_copy(
                                vws2v[:, jc, :, 0:KN],
                                vt_ps[:, jc * 128:(jc + 1) * 128].rearrange(
                                    "p (h e) -> p h e", h=2
                                ),
                            )
                        nc.vector.memset(vws2v[:, :, :, KN:KN + 1], 1.0)
                        # unnormalized o + exp row sums (col KN)
                        uo = psuo.tile([128, 2 * 2 * (KN + 1)], f32,
                                       tag="uo")
                        for hl in range(HL):
                            for ic in range(IC):
                                c0 = hl * 130 + ic * 65
                                for jc in range(2):
                                    nc.tensor.matmul(
                                        uo[:, c0:c0 + KN + 1],
                                        eT[:, hl * 512 + jc * 256 + ic * 128:
                                           hl * 512 + jc * 256 +
                                           (ic + 1) * 128],
                                        vws2v[:, jc, hl, :],
                                        start=(jc == 0), stop=(jc == 1),
                                    )
                        rec = smp.tile([128, 4], f32, tag="rec")
                        nc.vector.reciprocal(
                            rec[:].rearrange("p (h i e) -> p h i e",
                                             h=2, i=2),
                            uo[:].rearrange("p (h i e) -> p h i e",
                                            h=2, i=2)[:, :, :, KN:KN + 1],
                        )
                        for hl in range(HL):
                            for ic in range(IC):
                                c0 = hl * 130 + ic * 65
                                dst = Ot_v[:, ic, hl * 32 + b, :]
                                rc = rec[:, 2 * hl + ic:2 * hl + ic + 1]
                                if ic == 0 and hl == 0:
                                    nc.scalar.activation(
                                        dst, uo[:, c0:c0 + KN], AF.Identity,
                                        bias=0.0, scale=rc,
                                    )
                                else:
                                    nc.vector.tensor_scalar(
                                        dst, uo[:, c0:c0 + KN], rc, None,
                                        op0=OP.mult,
                                    )

            # ---------- Phase D: BN1 stats + apply ----------
            with (
                tc.tile_pool(name="st1", bufs=1) as st1,
                tc.tile_pool(name="psD", bufs=1, space="PSUM") as psD,
            ):
                npe = IC * 32 * KN     # 4096 elems/partition/head
                st2 = st1.tile([128, 4], f32, tag="st2")
                for hl in range(HL):
                    Ov = Ot_v[:, :, hl * 32:(hl + 1) * 32, :]
                    scrap = st1.tile([128, npe], bf16, tag=f"scrap{hl}",
                                     name=f"scrap{hl}")
                    sum1 = st1.tile([128, 1], f32, tag=f"sum{hl}")
                    nc.vector.tensor_scalar(
                        scrap[:].rearrange("p (a t k) -> p a t k",
                                           a=IC, t=32),
                        Ov, 1.0, None, op0=OP.mult, op1=OP.add,
                        accum_out=sum1[:],
                    )
                    scrap2 = st1.tile([128, npe], bf16, tag=f"scrap2{hl}",
                                      name=f"scrap2{hl}")
                    sq1 = st1.tile([128, 1], f32, tag=f"sq{hl}")
                    nc.scalar.activation(
                        scrap2[:].rearrange("p (a t k) -> p a t k",
                                            a=IC, t=32),
                        Ov, AF.Square, accum_out=sq1[:],
                    )
                    nc.vector.tensor_scalar_mul(
                        st2[:, 2 * hl:2 * hl + 1], sum1[:], 1.0 / npe
                    )
                    nc.vector.tensor_scalar_mul(
                        st2[:, 2 * hl + 1:2 * hl + 2], sq1[:], 1.0 / npe
                    )
                hs1 = psD.tile([1, 4], f32, tag="hs1")
                nc.tensor.matmul(hs1[:], ones128, st2[:])
                hsb = st1.tile([1, 4], f32, tag="hsb")
                nc.vector.tensor_copy(hsb[:], hs1[:])
                ab1 = st1.tile([1, 4], f32, tag="ab1")
                for hl in range(HL):
                    m_ = hsb[:, 2 * hl:2 * hl + 1]
                    e2 = hsb[:, 2 * hl + 1:2 * hl + 2]
                    m2 = st1.tile([1, 1], f32, tag=f"m2_{hl}")
                    nc.vector.tensor_tensor(m2[:], m_, m_, op=OP.mult)
                    var1 = st1.tile([1, 1], f32, tag=f"v1_{hl}")
                    nc.vector.tensor_tensor(var1[:], e2, m2[:],
                                            op=OP.subtract)
                    nc.vector.tensor_scalar_add(var1[:], var1[:], EPS)
                    rv1 = st1.tile([1, 1], f32, tag=f"rv1_{hl}")
                    nc.vector.reciprocal(rv1[:], var1[:])
                    rsq1 = st1.tile([1, 1], f32, tag=f"rsq1_{hl}")
                    nc.scalar.sqrt(rsq1[:], rv1[:])
                    nc.vector.tensor_tensor(
                        ab1[:, 2 * hl:2 * hl + 1],
                        bnp1_sb[:, 2 * hl:2 * hl + 1], rsq1[:], op=OP.mult,
                    )
                    t2 = st1.tile([1, 1], f32, tag=f"t2_{hl}")
                    nc.vector.tensor_tensor(
                        t2[:], m_, ab1[:, 2 * hl:2 * hl + 1], op=OP.mult
                    )
                    nc.vector.tensor_tensor(
                        ab1[:, 2 * hl + 1:2 * hl + 2],
                        bnp1_sb[:, 2 * hl + 1:2 * hl + 2], t2[:],
                        op=OP.subtract,
                    )
                bc2 = psD.tile([128, 4], f32, tag="bc2")
                nc.tensor.matmul(bc2[:], bc1_sb, ab1[:])
                a1b1 = st1.tile([128, 4], f32, tag="a1b1")
                nc.vector.tensor_copy(a1b1[:], bc2[:])
                # apply BN1 (transposing (t,k)->(k,t)), then AG per ic half
                flocs = []
                fgls = []
                for ic in range(IC):
                    for hl in range(HL):
                        dstv = O_v[:, ic, :, hl * 32:(hl + 1) * 32]
                        srcv = Ot_v[:, ic, hl * 32:(hl + 1) * 32, :] \
                            .transpose([0, 2, 1])
                        eng = nc.vector if hl == 0 else nc.gpsimd
                        eng.tensor_scalar(
                            dstv, srcv, a1b1[:, 2 * hl:2 * hl + 1],
                            a1b1[:, 2 * hl + 1:2 * hl + 2],
                            op0=OP.mult, op1=OP.add,
                        )
                    floc = dp.tile([128, FW // 2], bf16, tag=f"floc{ic}",
                                   name=f"floc{ic}")
                    nc.sync.dma_start(
                        floc[:], O_all[:, ic * 4096:(ic + 1) * 4096]
                    )
                    fgl = dp.tile([N_CORES, 128, FW // 2], bf16,
                                  tag=f"fgl{ic}", name=f"fgl{ic}",
                                  addr_space="Shared")
                    nc.gpsimd.collective_compute(
                        "AllGather", OP.bypass, replica_groups=RG,
                        ins=[floc[:].opt()], outs=[fgl[:].opt()],
                    )
                    flocs.append(floc)
                    fgls.append(fgl)
            otp_cm.__exit__(None, None, None)

            # ---------- Phase F: h1 shard, leaky via affine+max ----------
            with (
                tc.tile_pool(name="fglp", bufs=1) as fp,
                tc.tile_pool(name="w1p", bufs=3) as w1p,
                tc.tile_pool(name="h1sbp", bufs=1) as hp,
                tc.tile_pool(name="psH", bufs=1, space="PSUM") as psH,
            ):
                fgl_sb = fp.tile([128, N_CORES * FW], bf16, tag="fglsb")
                v1 = fgl_sb[:].rearrange("p (c x) -> p c x", c=N_CORES)
                nc.sync.dma_start(
                    v1[:, :, 0:4096],
                    fgls[0][:].transpose([1, 0, 2]),
                )
                h1ps = [
                    psH.tile([128, T], f32, tag=f"h1_{j}", name=f"h1ps{j}")
                    for j in range(5)
                ]
                w1prev = w1pre[:].rearrange("p (m j) -> p m j", m=NPRE)
                for m in range(NKT // 2):
                    if m == NPRE // 2 + 3:
                        nc.sync.dma_start(
                            v1[:, :, 4096:8192],
                            fgls[1][:].transpose([1, 0, 2]),
                        )
                    if 2 * m + 1 < NPRE:
                        w1ts = [w1prev[:, 2 * m, :], w1prev[:, 2 * m + 1, :]]
                    else:
                        w1tt = w1p.tile([128, 2 * HSH], bf16, tag="w1t",
                                        name=f"w1t{m}")
                        nc.sync.dma_start(
                            w1tt[:].rearrange("p (s j) -> p s j", s=2),
                            w1_d[2 * m:2 * m + 2].transpose([1, 0, 2]),
                        )
                        w1ts = [w1tt[:, 0:HSH], w1tt[:, HSH:2 * HSH]]
                    for s in range(2):
                        kt = 2 * m + s
                        w1t = w1ts[s]
                        ic, kk = divmod(kt, KN)
                        rhs = v1[:, :, ic * 4096:(ic + 1) * 4096].rearrange(
                            "p c (k t) -> p c k t", k=KN
                        )[:, :, kk, :]
                        for j in range(5):
                            nc.tensor.matmul(
                                h1ps[j][:],
                                w1t[:, j * 128:(j + 1) * 128],
                                rhs,
                                start=(kt == 0), stop=(kt == NKT - 1),
                            )
                h1sb = [
                    hp.tile([128, T], bf16, tag=f"h1s_{j}", name=f"h1s{j}")
                    for j in range(5)
                ]
                h1af = [
                    hp.tile([128, T], bf16, tag=f"h1a_{j}", name=f"h1a{j}")
                    for j in range(5)
                ]
                for j in range(5):
                    nc.scalar.activation(
                        h1af[j][:], h1ps[j][:], AF.Identity,
                        bias=b1_sb[j], scale=1.0,
                    )
                    nc.vector.scalar_tensor_tensor(
                        h1sb[j][:], h1af[j][:], SLOPE, h1af[j][:],
                        op0=OP.mult, op1=OP.max,
                    )

                # ---------- Phase G: W2 partial + AllReduce + sigmoid ---
                ps2 = psH.tile([KN, T], f32, tag="out2")
                for j in range(5):
                    nc.tensor.matmul(
                        ps2[:], w2_sb[j], h1sb[j][:],
                        start=(j == 0), stop=(j == 4),
                    )
                o2sb = hp.tile([KN, T], f32, tag="o2sb")
                nc.vector.tensor_copy(o2sb[:], ps2[:])
                arin = dp.tile([KN, T], f32, tag="arin")
                nc.sync.dma_start(arin[:], o2sb[:])
                arout = dp.tile([KN, T], f32, tag="arout",
                                addr_space="Shared")
                nc.gpsimd.collective_compute(
                    "AllReduce", OP.add, replica_groups=RG,
                    ins=[arin[:].opt()], outs=[arout[:].opt()],
                )
                arsb = hp.tile([KN, T], f32, tag="arsb")
                nc.sync.dma_start(arsb[:], arout[:])
                fin = hp.tile([KN, T], f32, tag="fin")
                nc.scalar.activation(
                    fin[:], arsb[:], AF.Sigmoid, bias=b2_sb, scale=1.0
                )
                nc.sync.dma_start(out_d, fin[:])

    nc.compile()
    return nc


def _dup_wT(W, c):
    W = np.asarray(W, np.float32)
    cols = [W[8 * c + ST * hl: 8 * c + ST * hl + KN, :].T for hl in range(HL)]
    return np.concatenate(cols, axis=1)


def _dup_b(b, c):
    b = np.asarray(b, np.float32)
    rows = [b[8 * c + ST * hl: 8 * c + ST * hl + KN] for hl in range(HL)]
    return np.ascontiguousarray(np.concatenate(rows))


def _prep_in_maps(inputs):
    import ml_dtypes

    f = np.float32
    bf = ml_dtypes.bfloat16
    q = np.asarray(inputs["q"], f)
    k = np.asarray(inputs["k"], f)
    v = np.asarray(inputs["v"], f)
    qh = np.ascontiguousarray(
        q[:, 0].transpose(1, 0, 2).reshape(FN, BS * SL).astype(bf))
    kh = np.ascontiguousarray(
        k[:, 0].transpose(1, 0, 2).reshape(FN, BS * SL).astype(bf))
    vh = np.ascontiguousarray(
        v[:, 0].transpose(1, 0, 2).reshape(FN, BS * SL).astype(bf))
    W1 = np.asarray(inputs["W1"], f)
    W1p = np.zeros((HIDP, SL * KN), f)
    W1p[:HID] = W1
    # device contraction row ((ic*64+kk)*128+p) = orig col ((ic*128+p)*64+kk)
    W1p = W1p.reshape(HIDP, IC, 128, KN).transpose(1, 3, 2, 0).reshape(
        SL * KN, HIDP)
    b1p = np.zeros((HIDP,), f)
    b1p[:HID] = np.asarray(inputs["b1"], f)
    W2T = np.zeros((HIDP, KN), f)
    W2T[:HID] = np.asarray(inputs["W2"], f).T
    b2 = np.asarray(inputs["b2"], f)
    in_maps = []
    for c in range(N_CORES):
        h0 = HL * c
        packf = np.zeros((128, PCW), f)
        packf[:, PC_BQ] = _dup_b(inputs["bq"], c)
        packf[:, PC_BK] = _dup_b(inputs["bk"], c)
        packf[:, PC_BV] = _dup_b(inputs["bv"], c)
        for hl in range(HL):
            packf[KN * hl:KN * (hl + 1), PC_MASK + hl] = 1.0 / KN
            packf[hl, PC_SEL + hl * KN:PC_SEL + (hl + 1) * KN] = 1.0
        b1c = b1p[c * HSH:(c + 1) * HSH]
        for j in range(5):
            packf[:, PC_B1 + j] = b1c[j * 128:(j + 1) * 128]
        packf[0:KN, PC_B2] = b2
        packf[:, PC_ONES] = 1.0 / 128.0
        packf[0, PC_BC1:PC_BC1 + 128] = 1.0
        for hl in range(HL):
            packf[hl, PC_BNP:PC_BNP + 8] = [
                inputs["gq"][h0 + hl], inputs["beq"][h0 + hl],
                inputs["gk"][h0 + hl], inputs["bek"][h0 + hl],
                inputs["gv"][h0 + hl], inputs["bev"][h0 + hl],
                inputs["g1"][h0 + hl], inputs["be1"][h0 + hl],
            ]
        packf[0, PC_BNP1:PC_BNP1 + 4] = [
            inputs["g1"][h0], inputs["be1"][h0],
            inputs["g1"][h0 + 1], inputs["be1"][h0 + 1],
        ]
        packb = np.zeros((128, 128 + 5 * KN), f)
        packb[:, 0:128] = np.eye(128, dtype=f)
        W2c = W2T[c * HSH:(c + 1) * HSH, :]
        for j in range(5):
            packb[:, 128 + j * KN:128 + (j + 1) * KN] = \
                W2c[j * 128:(j + 1) * 128, :]
        m = {
            "qh": qh, "kh": kh, "vh": vh,
            "wqT": np.ascontiguousarray(_dup_wT(inputs["Wq"], c).astype(bf)),
            "wkT": np.ascontiguousarray(_dup_wT(inputs["Wk"], c).astype(bf)),
            "wvT": np.ascontiguousarray(_dup_wT(inputs["Wv"], c).astype(bf)),
            "packf": packf,
            "packb": np.ascontiguousarray(packb.astype(bf)),
            "w1T": np.ascontiguousarray(
                W1p[:, c * HSH:(c + 1) * HSH]
                .reshape(NKT, 128, HSH).astype(bf)),
        }
        in_maps.append(m)
    return in_maps


def _unshard(o):
    out = (
        np.asarray(o, np.float32)
        .reshape(KN, N_CORES, HL, BS)
        .transpose(3, 1, 2, 0)
        .reshape(BS, HEADS, KN)[:, None]
    )
    return np.ascontiguousarray(out.astype(np.float32))


def kernel(**inputs):
    global _prog
    if _prog is None:
        _prog = _build()
    from concourse.bass_utils import run_bass_kernel_spmd

    in_maps = _prep_in_maps(inputs)
    res = run_bass_kernel_spmd(_prog, in_maps, list(range(N_CORES)))
    return _unshard(res.results[0]["out"])

